# revision 3
# baseline (speedup 1.0000x reference)
"""CrossEntropyLabelSmooth loss kernel for Trainium2 (8 NeuronCores, Bass/Tile).

Math reduction: with log_probs = x - lse(x) per row, the scalar loss equals

  loss = mean_i [ lse_i - WH*x[i,tgt_i] - WS*sum_j x[i,posvid_ij] - BC*sum_c x[i,c] ]

where
  WH = (1-w)(1-eps) + w(1-lam)      (hard-target gather weight)
  WS = w*lam/P                      (per-posvid gather weight)
  BC = (1-w)*eps/C                  (full-row-sum weight)

Device work per core (data-parallel over the batch dim, 512 rows/core):
  - one streaming pass over x: ACT exp with fused row-sum accumulation,
    DVE row sum of raw x
  - gathers via SWDGE dma_gather of 256B-aligned chunks + host-built sparse
    weight masks, fused multiply-reduce on DVE
  - per-row lse via ACT Ln, partition-reduction via a ones-vector matmul on PE
Host: shard inputs, build gather indices/weights (index-only preprocessing),
sum the 8 per-core scalar partials, divide by B.
"""
import sys

sys.path.insert(0, "/opt/trn_rl_repo")

import numpy as np

# Problem shapes (hardcoded per contract)
B, C, P = 4096, 32000, 50
N_CORES = 8
B_CORE = B // N_CORES            # 512 rows per core
RB = B_CORE // 128               # 4 row blocks of 128 partitions
W = 4000                         # column tile width
N_CT = C // W                    # 8 column tiles

CHUNK = 64                       # f32 elements per gather chunk (256B)
CPR = C // CHUNK                 # 500 chunks per row
SLAB = 64                        # rows per gather slab (int16 index range)
N_SLABS = B_CORE // SLAB         # 8
IDX_REAL = SLAB * (P + 1)        # 3264 gathers per slab
IDX_PAD = 3328                   # = 26 * 128
G = IDX_PAD // 128               # 26 gather groups
IDXW = IDX_PAD // 16             # 208 wrapped-index columns

EPSILON, SOFT_W, SOFT_LAM = 0.1, 0.1, 0.2
W_HARD = (1.0 - SOFT_W) * (1.0 - EPSILON) + SOFT_W * (1.0 - SOFT_LAM)  # 0.89
W_SOFT = SOFT_W * SOFT_LAM / P                                         # 4e-4
B_COEF = (1.0 - SOFT_W) * EPSILON / C                                  # 2.8125e-6

_CACHE = {}


def build_nc():
    if "nc" in _CACHE:
        return _CACHE["nc"]
    import concourse.bass as bass
    import concourse.bacc as bacc
    import concourse.tile as tile
    import concourse.mybir as mybir
    from contextlib import ExitStack

    f32 = mybir.dt.float32
    i16 = mybir.dt.int16

    nc = bacc.Bacc("TRN2", target_bir_lowering=False, debug=False)
    x_t = nc.dram_tensor("x", [B_CORE, C], f32, kind="ExternalInput")
    gix_t = nc.dram_tensor("gidx", [128, N_SLABS * IDXW], i16, kind="ExternalInput")
    gw_t = nc.dram_tensor("gw", [N_SLABS, 128, G, CHUNK], f32, kind="ExternalInput")
    out_t = nc.dram_tensor("out", [1, 1], f32, kind="ExternalOutput")

    with tile.TileContext(nc) as tc, ExitStack() as ctx:
        xpool = ctx.enter_context(tc.tile_pool(name="xp", bufs=3))
        epool = ctx.enter_context(tc.tile_pool(name="ep", bufs=2))
        gpool = ctx.enter_context(tc.tile_pool(name="gp", bufs=2))
        wpool = ctx.enter_context(tc.tile_pool(name="wp", bufs=2))
        jpool = ctx.enter_context(tc.tile_pool(name="jp", bufs=2))
        spool = ctx.enter_context(tc.tile_pool(name="sp", bufs=1))
        ppool = ctx.enter_context(
            tc.tile_pool(name="ps", bufs=1, space=bass.MemorySpace.PSUM)
        )

        esums = spool.tile([128, RB, N_CT], f32)
        xsums = spool.tile([128, RB, N_CT], f32)
        gsums = spool.tile([128, N_SLABS], f32)
        gix_sb = spool.tile([128, N_SLABS * IDXW], i16)
        nc.sync.dma_start(gix_sb[:], gix_t[:, :])

        # Main streaming pass over x
        for rb in range(RB):
            for ct in range(N_CT):
                t = xpool.tile([128, W], f32)
                nc.sync.dma_start(
                    t[:], x_t[rb * 128 : (rb + 1) * 128, ct * W : (ct + 1) * W]
                )
                eo = epool.tile([128, W], f32)
                nc.scalar.activation(
                    eo[:],
                    t[:],
                    mybir.ActivationFunctionType.Exp,
                    accum_out=esums[:, rb : rb + 1, ct : ct + 1],
                )
                nc.vector.tensor_reduce(
                    xsums[:, rb : rb + 1, ct : ct + 1],
                    t[:],
                    axis=mybir.AxisListType.X,
                    op=mybir.AluOpType.add,
                )

        # Gather pass: 8 slabs of 64 rows, 256B-chunk gathers + weighted dot
        for s in range(N_SLABS):
            wt = wpool.tile([128, G, CHUNK], f32)
            nc.sync.dma_start(wt[:], gw_t[s])
            ga = gpool.tile([128, G, CHUNK], f32)
            in_ap = bass.AP(x_t, s * SLAB * C, [[CHUNK, SLAB * CPR], [1, CHUNK]])
            nc.gpsimd.dma_gather(
                ga[:],
                in_ap,
                gix_sb[:, s * IDXW : (s + 1) * IDXW],
                num_idxs=IDX_PAD,
                num_idxs_reg=IDX_PAD,
                elem_size=CHUNK,
                single_packet=False,
            )
            ju = jpool.tile([128, G, CHUNK], f32)
            nc.vector.tensor_mul(ju[:], ga[:], wt[:])
            nc.vector.tensor_reduce(
                gsums[:, s : s + 1],
                ju[:],
                axis=mybir.AxisListType.XY,
                op=mybir.AluOpType.add,
            )

        # Finale: lse per row, combine all partial sums, reduce over partitions
        sexp = spool.tile([128, RB], f32)
        nc.vector.tensor_reduce(
            sexp[:], esums[:], axis=mybir.AxisListType.X, op=mybir.AluOpType.add
        )
        lse = spool.tile([128, RB], f32)
        nc.scalar.activation(lse[:], sexp[:], mybir.ActivationFunctionType.Ln)
        lsum = spool.tile([128, 1], f32)
        nc.vector.tensor_reduce(
            lsum[:], lse[:], axis=mybir.AxisListType.X, op=mybir.AluOpType.add
        )
        xtot = spool.tile([128, 1], f32)
        nc.vector.tensor_reduce(
            xtot[:], xsums[:], axis=mybir.AxisListType.XY, op=mybir.AluOpType.add
        )
        gtot = spool.tile([128, 1], f32)
        nc.vector.tensor_reduce(
            gtot[:], gsums[:], axis=mybir.AxisListType.X, op=mybir.AluOpType.add
        )
        # part = lsum - B_COEF*xtot - gtot
        xs = spool.tile([128, 1], f32)
        nc.vector.tensor_scalar_mul(xs[:], xtot[:], -B_COEF)
        p1 = spool.tile([128, 1], f32)
        nc.vector.tensor_sub(p1[:], lsum[:], gtot[:])
        part = spool.tile([128, 1], f32)
        nc.vector.tensor_add(part[:], p1[:], xs[:])
        ones = spool.tile([128, 1], f32)
        nc.vector.memset(ones[:], 1.0)
        ps = ppool.tile([1, 1], f32)
        nc.tensor.matmul(ps[:], ones[:], part[:], start=True, stop=True)
        res = spool.tile([1, 1], f32)
        nc.vector.tensor_copy(res[:], ps[:])
        nc.sync.dma_start(out_t[:, :], res[:])

    nc.compile()
    _CACHE["nc"] = nc
    return nc


def _host_prep(targets, all_posvid):
    """Build per-core gather index (int16, wrapped) and dense weight tensors.

    Index-only preprocessing: never touches the values of `inputs`.
    """
    tg = np.asarray(targets).astype(np.int64).reshape(B)
    pv = np.asarray(all_posvid).astype(np.int64).reshape(B, P)
    cols = np.concatenate([tg[:, None], pv], axis=1)               # [B, 51]
    wts = np.concatenate(
        [
            np.full((B, 1), W_HARD, np.float32),
            np.full((B, P), W_SOFT, np.float32),
        ],
        axis=1,
    )                                                               # [B, 51]
    rloc = np.arange(B) % SLAB
    chunk = rloc[:, None] * CPR + cols // CHUNK                     # [B, 51]
    off = (cols % CHUNK).astype(np.int64)                           # [B, 51]

    jj = np.arange(IDX_REAL)
    p_idx = jj % 128
    g_idx = jj // 128

    gidx_cores = []
    gw_cores = []
    for c in range(N_CORES):
        gixs = np.zeros((N_SLABS, 128, IDXW), np.int16)
        gws = np.zeros((N_SLABS, 128, G, CHUNK), np.float32)
        for s in range(N_SLABS):
            r0 = c * B_CORE + s * SLAB
            ch_flat = chunk[r0 : r0 + SLAB].reshape(-1)             # [3264]
            off_flat = off[r0 : r0 + SLAB].reshape(-1)
            wt_flat = wts[r0 : r0 + SLAB].reshape(-1)
            idx16 = np.zeros(IDX_PAD, np.int16)
            idx16[:IDX_REAL] = ch_flat.astype(np.int16)
            # wrapped layout: index j lives at partition j%16, column j//16
            gixs[s] = np.tile(idx16.reshape(IDXW, 16).T, (8, 1))
            gws[s, p_idx, g_idx, off_flat] = wt_flat
        gidx_cores.append(
            np.ascontiguousarray(gixs.transpose(1, 0, 2).reshape(128, N_SLABS * IDXW))
        )
        gw_cores.append(gws)
    return gidx_cores, gw_cores


def kernel(inputs, targets, all_posvid):
    from concourse.bass_utils import run_bass_kernel_spmd

    nc = build_nc()
    x = np.ascontiguousarray(np.asarray(inputs, dtype=np.float32).reshape(B, C))
    gidx_cores, gw_cores = _host_prep(targets, all_posvid)

    in_maps = [
        {
            "x": x[c * B_CORE : (c + 1) * B_CORE],
            "gidx": gidx_cores[c],
            "gw": gw_cores[c],
        }
        for c in range(N_CORES)
    ]
    res = run_bass_kernel_spmd(nc, in_maps, core_ids=list(range(N_CORES)))
    total = np.float64(0.0)
    for c in range(N_CORES):
        total += np.float64(res.results[c]["out"][0, 0])
    return np.float32(total / B)
